# revision 1
# baseline (speedup 1.0000x reference)
"""Causal multi-head self-attention with RoPE on 8 Trainium2 NeuronCores.

Sharding: data-parallel over batch (4) x tensor-parallel over heads (2 groups
of 8 heads). Core c handles batch c//2, head group c%2. Each core computes a
partial output projection y_part = attnout_g @ Wo_g.T; the host sums the two
partials per batch.

Device algorithm (per core), all matmul operands bf16, f32 PSUM accumulation:
  1. Projections blocked by 512-query chunk with the model-dim (kt) loop
     OUTER so the first PSUM group retires after one kt tile arrives; DMAs
     are issued in consumption order so PE starts ~1us in.
  2. RoPE pairs are laid out as a +-16 partition swap within each 32-block
     (host-side head-dim permutation), so the partner shuffle is a single
     DVE stream_shuffle - no SBUF-to-SBUF DMA.
  3. Attention per 512-query chunk, one head-pair per group: scoresT[j,i]
     via K=64 matmuls with trapezoid column slicing, exp on ACT (scale
     folded, no max-subtract: |scaled scores| < ~10), causal 0/1 triangle
     mask (one shared [128,128] table) multiplied on DVE over just the
     128 diagonal columns, PV + Z via M=65 matmuls accumulated over key
     tiles. PV results copy straight from PSUM into outUT via DVE
     cross-quadrant writes.
  4. Output projection groups accumulate in their own 2-bank PSUM pool and
     are emitted interleaved into the attention jt loops as PE filler.
"""

import sys

if "/opt/trn_rl_repo" not in sys.path:
    sys.path.insert(0, "/opt/trn_rl_repo")

import numpy as np
import ml_dtypes

import concourse.bass as bass
import concourse.bacc as bacc
import concourse.tile as tile
import concourse.mybir as mybir
from concourse.bass_utils import run_bass_kernel_spmd

BF16 = ml_dtypes.bfloat16
F32 = mybir.dt.float32
BF = mybir.dt.bfloat16

B, S, D = 4, 2048, 1024
H, DK = 16, 64
HL = 8          # heads per core
NCORES = 8
THETA = 10000.0
SCALE = 1.0 / 8.0  # 1/sqrt(64)
P = 128
KT_D = D // P   # 8 k-tiles over model dim
NMT = 4         # QT/KT partition tiles (512 head dims / 128)
ST = S // P     # 16 s-tiles
IC = S // 512   # 4 query chunks of 512

SHUF16 = [(i + 16) % 32 for i in range(32)]


def emit_program(nc, prm, reps=1):
    """Emit the whole per-core program. prm maps name -> DRAM AP."""
    xT = prm["xT"].rearrange("(kt p) s -> p kt s", p=P)        # [128, 8, 2048]
    wqT = prm["wqT"].rearrange("(kt p) o -> p kt o", p=P)      # [128, 8, 512]
    wkT = prm["wkT"].rearrange("(kt p) o -> p kt o", p=P)
    wvT = prm["wvT"].rearrange("(kt p) o -> p kt o", p=P)
    woT = prm["woT"].rearrange("(kt p) o -> p kt o", p=P)      # [128, 4, 1024]
    cosT = prm["cosT"]                                         # [128, 2048] f32
    sinT = prm["sinT"]
    maskT = prm["maskT"]                                       # [128, 128] bf16
    y = prm["y"].rearrange("(st p) o -> p st o", p=P)          # [128, 16, 1024]

    with tile.TileContext(nc) as tc:
        import contextlib
        ctx = contextlib.ExitStack()
        with ctx:
            # --- persistent pools (loaded once; reps model steady state) ---
            persist = ctx.enter_context(tc.tile_pool(name="persist", bufs=1))
            dram = ctx.enter_context(tc.tile_pool(name="dram", bufs=1, space="DRAM"))

            qrot = persist.tile([P, NMT, S], BF, tag="qrot")
            krot = persist.tile([P, NMT, S], BF, tag="krot")
            vab = persist.tile([P, ST, HL * 65], BF, tag="vab")
            masks = persist.tile([P, 128], BF, tag="masks")
            wo_s = persist.tile([P, 4, D], BF, tag="wo_s")
            wq_s = persist.tile([P, KT_D, 512], BF, tag="wq_s")
            wk_s = persist.tile([P, KT_D, 512], BF, tag="wk_s")
            wv_s = persist.tile([P, KT_D, 512], BF, tag="wv_s")
            cos_s = persist.tile([P, S], F32, tag="cos_s")
            sin_s = persist.tile([P, S], F32, tag="sin_s")

            ones_r = persist.tile([P, P], BF, tag="ones_r")
            nc.vector.memset(ones_r[0:1, :], 1.0)

            # attention staging persists so chunk 0 can run inside phase 1
            outUT = persist.tile([P, NMT, S], BF, tag="outUT")
            outT = persist.tile([P, NMT, S], BF, tag="outT")

            def mask_bc(o):
                # [128, 2, 128] view of the shared triangle, bcast over dim 1
                m = masks[:, 0:128]
                return bass.AP(tensor=m.tensor, offset=m.offset,
                               ap=[m.ap[0], [0, 2], m.ap[1]])

            # ones column per head block (col 64 of each 65-wide block)
            vab_heads = vab.rearrange("p st (h c) -> p st h c", c=65)
            nc.vector.memset(vab_heads[:, :, :, 64], 1.0)

            def make_pair_steps(rep, ic, hp, psA, psPV, pexp, pzrow, zrows,
                                filler=None):
                """Micro-step callables for one head-pair of chunk ic: one
                per jt plus a drain step."""
                i0 = ic * 512
                jmax = 4 * ic + 3
                heads = (2 * hp, 2 * hp + 1)
                pvs = {h: psPV.tile([65, 512], F32, tag="pv",
                                    name=f"pv_{rep}_{ic}_{h}")
                       for h in heads}
                es = {}

                def off(jt):
                    v = jt - 4 * ic
                    return 128 * v if v > 0 else 0

                def emit_pv(jt):
                    o = off(jt)
                    e = es.pop(jt)
                    for idx, h in enumerate(heads):
                        nc.tensor.matmul(
                            pvs[h][:, o:512],
                            lhsT=vab_heads[:, jt, h, :],
                            rhs=e[:, idx, o:512],
                            start=(jt == 0), stop=(jt == jmax),
                        )

                def jt_step(jt):
                    o = off(jt)
                    ps = psA.tile([P, 2, 512], F32, tag="sc",
                                  name=f"sc_{rep}_{ic}_{hp}_{jt}")
                    for idx, h in enumerate(heads):
                        po = idx * 64
                        nc.tensor.matmul(
                            ps[:, idx, o:512],
                            lhsT=krot[po:po + 64, hp, jt * P:(jt + 1) * P],
                            rhs=qrot[po:po + 64, hp, i0 + o:i0 + 512],
                            start=True, stop=True,
                        )
                    e = pexp.tile([P, 2, 512], BF, tag="e")
                    nc.scalar.activation(
                        out=e[:, :, o:512], in_=ps[:, :, o:512],
                        func=mybir.ActivationFunctionType.Exp,
                        scale=SCALE,
                    )
                    if jt - 4 * ic >= 0:
                        nc.vector.tensor_mul(
                            e[:, :, o:o + 128], e[:, :, o:o + 128],
                            mask_bc(o))
                    es[jt] = e
                    # filler goes BEFORE the pv so pv waits (exp / previous
                    # pair draining) hide behind the filler work
                    if filler is not None:
                        filler()
                    # pv trails scores by 2 steps so ACT's exp latency and
                    # jitter never stall the PE
                    if jt > 1:
                        emit_pv(jt - 2)

                def drain():
                    if jmax - 1 in es:  # predrain not used (unmerged path)
                        emit_pv(jmax - 1)
                    emit_pv(jmax)
                    # drain the pair's PSUM: outU rows straight into outUT,
                    # Z row to partition 0 (DVE cross-quadrant writes)
                    for idx, h in enumerate(heads):
                        po = idx * 64
                        nc.vector.tensor_copy(
                            out=outUT[po:po + 64, hp, i0:i0 + 512],
                            in_=pvs[h][0:64, :])
                        zrow = pzrow.tile([1, 512], BF, tag="zrow",
                                          name=f"zrow_{rep}_{ic}_{h}")
                        nc.vector.tensor_copy(
                            out=zrow, in_=pvs[h][64:65, :])
                        zrows[(h, ic)] = zrow

                def predrain():
                    # pv(jmax-1): emitted between the successor pair's first
                    # two steps so exp(jmax-1) has time to land
                    emit_pv(jmax - 1)

                steps = [lambda jt=jt: jt_step(jt) for jt in range(jmax + 1)]
                return steps, predrain, drain

            def emit_normalize(rep, ic, zrows, psZ, psZ_tag, precip):
                # broadcast Z over the 64 partitions of each head via rank-1
                # PE matmuls, then outT = outUT * (1/Z)
                i0 = ic * 512
                for ht in range(NMT):
                    zb = psZ.tile([P, 512], F32, tag=psZ_tag,
                                  name=f"zb_{rep}_{ic}_{ht}")
                    for idx in range(2):
                        nc.tensor.matmul(
                            zb[idx * 64:(idx + 1) * 64, :],
                            lhsT=ones_r[0:1, 0:64],
                            rhs=zrows[(2 * ht + idx, ic)],
                            start=True, stop=True)
                    recip = precip.tile([P, 512], F32, tag="recip")
                    nc.vector.reciprocal(out=recip, in_=zb)
                    nc.vector.tensor_mul(
                        outT[:, ht, i0:i0 + 512],
                        outUT[:, ht, i0:i0 + 512], recip)

            ypend = []   # carried across reps: last chunk's out-proj groups
            pending_norm = []  # deferred last-chunk normalize (next rep)
            zrows = {}
            pzrow = ctx.enter_context(tc.tile_pool(name="pzrow", bufs=18))
            precip = ctx.enter_context(tc.tile_pool(name="precip", bufs=4))
            for rep in range(reps):
                if True:
                    # ------------ phase 1: projections + rope + chunk 0 ------
                    with tc.tile_pool(name="ph1", bufs=1) as ph1, \
                         tc.tile_pool(name="praw", bufs=3) as praw, \
                         tc.tile_pool(name="pshuf", bufs=4) as pshuf, \
                         tc.tile_pool(name="pexp0", bufs=3) as pexp0, \
                         tc.tile_pool(name="pyst1", bufs=2) as pyst1, \
                         tc.tile_pool(name="psP", bufs=4, space="PSUM") as psP, \
                         tc.tile_pool(name="psA0", bufs=1, space="PSUM") as psA0, \
                         tc.tile_pool(name="psPV0", bufs=2, space="PSUM") as psPV0:
                        xts = ph1.tile([P, KT_D, S], BF, tag="xts")

                        # DMA issue order == consumption order: Q(qb0) paces
                        # on wq+x(qb0); K(qb0) on wk (interleaved with the
                        # cos/sin chunk 0 tables rope needs to free Q's
                        # PSUM); V(qb0) on wv; later qbs only need their x.
                        for kt in range(KT_D):
                            if rep == 0:
                                nc.sync.dma_start(out=wq_s[:, kt, :],
                                                  in_=wqT[:, kt, :])
                            nc.sync.dma_start(out=xts[:, kt, 0:512],
                                              in_=xT[:, kt, 0:512])
                        if rep == 0:
                            for kt in range(KT_D):
                                nc.sync.dma_start(out=wk_s[:, kt, :],
                                                  in_=wkT[:, kt, :])
                                if kt == 1:
                                    nc.sync.dma_start(out=cos_s[:, 0:512],
                                                      in_=cosT[:, 0:512])
                                if kt == 3:
                                    nc.sync.dma_start(out=sin_s[:, 0:512],
                                                      in_=sinT[:, 0:512])
                            for kt in range(KT_D):
                                nc.sync.dma_start(out=wv_s[:, kt, :],
                                                  in_=wvT[:, kt, :])
                                if kt == 1:
                                    nc.sync.dma_start(out=masks, in_=maskT)
                        for cc in range(1, 4):
                            for kt in range(KT_D):
                                nc.sync.dma_start(
                                    out=xts[:, kt, cc * 512:(cc + 1) * 512],
                                    in_=xT[:, kt, cc * 512:(cc + 1) * 512])
                            if rep == 0:
                                nc.sync.dma_start(
                                    out=cos_s[:, cc * 512:(cc + 1) * 512],
                                    in_=cosT[:, cc * 512:(cc + 1) * 512])
                                nc.sync.dma_start(
                                    out=sin_s[:, cc * 512:(cc + 1) * 512],
                                    in_=sinT[:, cc * 512:(cc + 1) * 512])
                        if rep == 0:
                            nc.sync.dma_start(out=wo_s, in_=woT)

                        def rope_apply(rot, mt, c0, ps):
                            # rot = ps*cos + shuffle16(ps*sinT); sin sign
                            # folded on host so the multiply is pre-shuffle
                            raw = praw.tile([P, 512], BF, tag="raw")
                            shuf = pshuf.tile([P, 512], BF, tag="shuf")
                            shufd = pshuf.tile([P, 512], BF, tag="shufd")
                            # both PSUM readers first so the slot frees for
                            # the next projection group one DVE op earlier
                            nc.vector.tensor_mul(raw, ps,
                                                 cos_s[:, c0:c0 + 512])
                            nc.vector.tensor_mul(shuf, ps,
                                                 sin_s[:, c0:c0 + 512])
                            nc.vector.stream_shuffle(shufd, shuf, mask=SHUF16)
                            nc.gpsimd.tensor_add(rot[:, mt, c0:c0 + 512],
                                                 raw, shufd)

                        # leftover out-proj groups from the previous rep's
                        # last chunk drain here, interleaved with qb0; the
                        # chunk-0 attention interleaves into qb1..3 (its
                        # inputs exist only after qb0)
                        def emit_y_group_ph1():
                            st, oc = ypend.pop(0)
                            ps = psP.tile([P, 512], F32, tag="psP",
                                          name=f"yp1_{rep}_{st}_{oc}")
                            for ht in range(NMT):
                                nc.tensor.matmul(
                                    ps,
                                    lhsT=outT[:, ht, st * P:(st + 1) * P],
                                    rhs=wo_s[:, ht, oc * 512:(oc + 1) * 512],
                                    start=(ht == 0), stop=(ht == NMT - 1),
                                )
                            ys = pyst1.tile([P, 512], F32, tag="ys")
                            nc.vector.tensor_copy(out=ys, in_=ps)
                            nc.sync.dma_start(
                                out=y[:, st, oc * 512:(oc + 1) * 512], in_=ys)

                        c0steps = []
                        for hp in range(4):
                            steps, predrain, drain = make_pair_steps(
                                rep, 0, hp, psA0, psPV0, pexp0, pzrow, zrows)
                            c0steps += steps + [predrain, drain]
                        c0n = [0]

                        def tick(allow_c0=True):
                            # pop leftover y / chunk-0 steps every other slot
                            if pending_norm:
                                return
                            c0n[0] += 1
                            while c0n[0] >= 2 and (
                                    ypend or (allow_c0 and c0steps)):
                                c0n[0] -= 2
                                if ypend:
                                    emit_y_group_ph1()
                                else:
                                    c0steps.pop(0)()

                        for qb in range(4):
                            c0 = qb * 512
                            for wt, rot in ((wq_s, qrot), (wk_s, krot)):
                                for sub in range(2):
                                    mts = (2 * sub, 2 * sub + 1)
                                    pss = {mt: psP.tile(
                                        [P, 512], F32, tag="psP",
                                        name=f"p1_{rep}_{qb}_{id(wt)}_{mt}")
                                        for mt in mts}
                                    for kt in range(KT_D):
                                        for mt in mts:
                                            nc.tensor.matmul(
                                                pss[mt],
                                                lhsT=wt[:, kt,
                                                        mt * P:(mt + 1) * P],
                                                rhs=xts[:, kt, c0:c0 + 512],
                                                start=(kt == 0),
                                                stop=(kt == KT_D - 1),
                                            )
                                        if kt % 2 == 1:
                                            tick(allow_c0=(qb > 0))
                                    for mt in mts:
                                        rope_apply(rot, mt, c0, pss[mt])
                                    # the previous rep's last-chunk normalize
                                    # lands here: its PE/DVE chain hides
                                    # behind this sub-round's matmuls
                                    while pending_norm:
                                        pic = pending_norm.pop(0)
                                        emit_normalize(pic[0], pic[1], zrows,
                                                       psP, "psP", precip)
                            # V projection for seq tiles of this block
                            for half in range(4):
                                st = 4 * qb + half
                                ps = psP.tile([P, 512], F32, tag="psP",
                                              name=f"psv_{rep}_{st}")
                                for kt in range(KT_D):
                                    nc.tensor.matmul(
                                        ps,
                                        lhsT=xts[:, kt, st * P:(st + 1) * P],
                                        rhs=wv_s[:, kt, :],
                                        start=(kt == 0),
                                        stop=(kt == KT_D - 1),
                                    )
                                if qb > 0:
                                    tick()
                                nc.scalar.copy(
                                    out=vab_heads[:, st, :, 0:64],
                                    in_=ps.rearrange("p (h c) -> p h c", c=64),
                                )
                        while ypend:
                            emit_y_group_ph1()
                        while c0steps:
                            c0steps.pop(0)()
                        emit_normalize(rep, 0, zrows, psP, "psP", precip)

                    # ------- phase 2: chunks 1-3 + normalize + out-proj ------
                    with tc.tile_pool(name="pexp", bufs=6) as pexp, \
                         tc.tile_pool(name="psA2", bufs=2, space="PSUM") as psA, \
                         tc.tile_pool(name="psPV", bufs=3, space="PSUM") as psPV, \
                         tc.tile_pool(name="psY", bufs=1, space="PSUM") as psY, \
                         tc.tile_pool(name="pyst", bufs=4) as pyst:
                        ypend.extend(
                            (st, oc) for st in range(4) for oc in range(2))

                        ycur = {}

                        def y_unit():
                            # one out-proj matmul (quarter of a y group):
                            # fine-grained PE filler so ACT's exp never
                            # starves the PE of ready work for long
                            if not ycur:
                                if not ypend:
                                    return
                                st, oc = ypend.pop(0)
                                ycur.update(
                                    st=st, oc=oc, ht=0,
                                    ps=psY.tile([P, 512], F32, tag="psYzb",
                                                name=f"yps_{rep}_{st}_{oc}"))
                            st, oc, ht, ps = (ycur["st"], ycur["oc"],
                                              ycur["ht"], ycur["ps"])
                            nc.tensor.matmul(
                                ps,
                                lhsT=outT[:, ht, st * P:(st + 1) * P],
                                rhs=wo_s[:, ht, oc * 512:(oc + 1) * 512],
                                start=(ht == 0), stop=(ht == NMT - 1),
                            )
                            if ht == NMT - 1:
                                ys = pyst.tile([P, 512], F32, tag="ys")
                                nc.vector.tensor_copy(out=ys, in_=ps)
                                nc.sync.dma_start(
                                    out=y[:, st, oc * 512:(oc + 1) * 512],
                                    in_=ys)
                                ycur.clear()
                            else:
                                ycur["ht"] = ht + 1

                        def emit_y_group():
                            y_unit()
                            while ycur:
                                y_unit()

                        def make_pacer(ic):
                            # spread the 32 queued y-units evenly over the
                            # chunk's (4ic+4)*4 jt-steps so PE filler is
                            # available whenever ACT's exp lags
                            steps_total = (4 * ic + 4) * 4
                            state = [0]

                            def f():
                                state[0] += 32
                                while state[0] >= steps_total and (
                                        ypend or ycur):
                                    state[0] -= steps_total
                                    y_unit()
                            return f

                        # software-pipeline consecutive pairs: each pair's
                        # first two jt-steps are emitted inside the previous
                        # pair's tail so ACT's exp pipeline never drains and
                        # the new pair's PV-slot wait hides under real work
                        seqs = []
                        for ic in range(1, IC):
                            pacer = make_pacer(ic)
                            for hp in range(4):
                                js, predrain, drain = make_pair_steps(
                                    rep, ic, hp, psA, psPV, pexp, pzrow,
                                    zrows, filler=pacer)
                                post = None
                                if hp == 3 and ic < IC - 1:
                                    def post(ic=ic):
                                        emit_normalize(rep, ic, zrows, psY,
                                                       "psYzb", precip)
                                        ypend.extend(
                                            (st, oc)
                                            for st in range(4 * ic, 4 * ic + 4)
                                            for oc in range(2))
                                elif hp == 3:
                                    # last chunk: defer normalize + out-proj
                                    # into the next rep's phase 1, where the
                                    # DVE chain hides behind projections
                                    def post(ic=ic):
                                        pending_norm.append((rep, ic))
                                        ypend.extend(
                                            (st, oc)
                                            for st in range(4 * ic, 4 * ic + 4)
                                            for oc in range(2))
                                seqs.append((js, predrain, drain, post))

                        # order per pair k (m = last jt):
                        #   js_k[2..m-1], j0_{k+1}, js_k[m], predrain_k,
                        #   j1_{k+1}, drain_k, post_k -- the successor's
                        #   steps hide pv/copy waits of the draining pair
                        for k, (js, predrain, drain, post) in enumerate(seqs):
                            start = 2 if k > 0 else 0
                            for idx in range(start, len(js) - 1):
                                js[idx]()
                            if k + 1 < len(seqs):
                                seqs[k + 1][0][0]()
                            js[-1]()
                            predrain()
                            if k + 1 < len(seqs):
                                seqs[k + 1][0][1]()
                            drain()
                            if post is not None:
                                post()

                        # the final rep has no successor phase 1 to drain into
                        if rep == reps - 1:
                            while pending_norm:
                                pic = pending_norm.pop(0)
                                emit_normalize(pic[0], pic[1], zrows, psY,
                                               "psYzb", precip)
                            while ypend:
                                emit_y_group()
    return nc


def build_nc(reps=1):
    nc = bacc.Bacc("TRN2", target_bir_lowering=False, debug=False,
                   num_devices=NCORES)
    prm = {}
    prm["xT"] = nc.declare_dram_parameter("xT", [D, S], BF, isOutput=False).ap()
    prm["wqT"] = nc.declare_dram_parameter("wqT", [D, 512], BF, isOutput=False).ap()
    prm["wkT"] = nc.declare_dram_parameter("wkT", [D, 512], BF, isOutput=False).ap()
    prm["wvT"] = nc.declare_dram_parameter("wvT", [D, 512], BF, isOutput=False).ap()
    prm["woT"] = nc.declare_dram_parameter("woT", [512, D], BF, isOutput=False).ap()
    prm["cosT"] = nc.declare_dram_parameter("cosT", [P, S], F32, isOutput=False).ap()
    prm["sinT"] = nc.declare_dram_parameter("sinT", [P, S], F32, isOutput=False).ap()
    prm["maskT"] = nc.declare_dram_parameter("maskT", [P, 128], BF,
                                             isOutput=False).ap()
    prm["y"] = nc.declare_dram_parameter("y", [S, D], F32, isOutput=True).ap()
    emit_program(nc, prm, reps=reps)
    nc.compile()
    return nc


def host_prep(x, token_positions):
    """Shared host-side layout prep. Returns cos/sin/mask tables."""
    pos = np.asarray(token_positions).astype(np.float32)
    p = np.arange(P)
    pp = p % 64
    # pair index per partition: 16 pairs per 32-block, partner at +-16
    i_freq = (pp % 16) + 16 * (pp // 32)
    freq = THETA ** (-i_freq / 32.0)                  # [128]
    freqs = pos[None, :] * freq[:, None]              # [128, S]
    cosT = np.cos(freqs).astype(np.float32).copy()
    # sign belongs to the SOURCE partition of the shuffle: +sin where the
    # even element of the pair lives (j<16), -sin where the odd lives
    sgn = np.where((p % 32) < 16, 1.0, -1.0).astype(np.float32)
    sinT = (np.sin(freqs) * sgn[:, None]).astype(np.float32).copy()

    j = np.arange(P)[:, None]
    i = np.arange(128)[None, :]
    maskT = (i >= j).astype(BF16)                     # [128, 128]
    return cosT, sinT, maskT


_NC_CACHE = {}


def _perm16():
    # within-head partition -> head-dim: 32-block b holds pairs 16b..16b+15,
    # even element at j, odd at j+16 (j = partition % 32 within the block)
    q = np.empty(64, np.int64)
    q[0:16] = 2 * np.arange(16)
    q[16:32] = 2 * np.arange(16) + 1
    q[32:48] = 2 * np.arange(16, 32)
    q[48:64] = 2 * np.arange(16, 32) + 1
    return q


def make_in_maps(x, token_positions, Wq, Wk, Wv, Wo):
    x = np.asarray(x)
    Wq, Wk, Wv, Wo = (np.asarray(a) for a in (Wq, Wk, Wv, Wo))
    cosT, sinT, maskT = host_prep(x, token_positions)

    perm = _perm16()
    WqT, WkT, WvT, WoT = Wq.T, Wk.T, Wv.T, Wo.T
    in_maps = []
    for c in range(NCORES):
        b, g = c // 2, c % 2
        colidx = np.concatenate([g * 512 + hl * 64 + perm for hl in range(HL)])
        in_maps.append({
            "xT": np.ascontiguousarray(x[b].T).astype(BF16),
            "wqT": np.ascontiguousarray(WqT[:, colidx]).astype(BF16),
            "wkT": np.ascontiguousarray(WkT[:, colidx]).astype(BF16),
            "wvT": np.ascontiguousarray(WvT[:, g * 512:(g + 1) * 512]).astype(BF16),
            "woT": np.ascontiguousarray(WoT[g * 512:(g + 1) * 512, :]).astype(BF16),
            "cosT": cosT,
            "sinT": sinT,
            "maskT": maskT,
        })
    return in_maps


def kernel(x, token_positions, Wq, Wk, Wv, Wo):
    in_maps = make_in_maps(x, token_positions, Wq, Wk, Wv, Wo)

    if "nc" not in _NC_CACHE:
        _NC_CACHE["nc"] = build_nc()
    nc = _NC_CACHE["nc"]

    res = run_bass_kernel_spmd(nc, in_maps, core_ids=list(range(NCORES)))
    y = np.zeros((B, S, D), np.float32)
    for c in range(NCORES):
        y[c // 2] += res.results[c]["y"]
    return y

